# Initial kernel scaffold
#
"""Trainium2 Bass kernel for MeshUp message-passing block.

reference:
    e_enc = (-e_rel) @ We.T + be                      # [N,512]
    msg   = concat([e_enc, v[idx], v_skip], -1)       # [N,1024]
    h     = LayerNorm(msg) * g + b
    h     = selu(h @ W0.T + b0); selu(h @ W1.T + b1); selu(h @ W2.T + b2);
    out   = selu(h @ W3.T + b3)                       # [N,512]

Strategy (per core, N_hr sharded 8 ways, v table replicated):
  - activations live as [feat, node] tiles (feat chunks of 128 on partitions,
    nodes on the free dim).  Layers 1-2 then need no transposes at all.
  - the e_enc half of msg plus the whole LayerNorm affine plus b0 fold into
    layer 0's matmul as a rank-4 (K=4) correction:
        z0 = W0g_v @ (v_cat * rstd)  +  [p1;p0;q;c0]^T @ [-r*rstd; rstd; -mu*rstd; 1]
    so e_enc is never materialized and LN costs only a stats pass (bn_stats
    on the gathered features + closed-form quadratic for the e_enc half).
  - selu(z) = L*relu(z) + min(L*A*exp(z), L*A) - L*A with the -L*A and L
    folded into the next layer's weights/biases -> 1 ACT pass (exp), 1 relu
    pass, 1 fused scalar_tensor_tensor per element.
  - layer 3 uses h3 as the matmul stationary operand so its output lands
    [node, feat] and DMAs straight to DRAM.
  - matmuls in bf16 (1 cyc/row on PE), stats + final output in fp32.
"""

import math
import os
import sys

for _p in ("/opt/trn_rl_repo", "/root/.axon_site/_ro/trn_rl_repo"):
    if os.path.isdir(_p) and _p not in sys.path:
        sys.path.insert(0, _p)

import numpy as np
import ml_dtypes

import concourse.bacc as bacc
import concourse.tile as tile
from concourse import library_config, mybir
from concourse.bass_utils import run_bass_kernel_spmd

F32 = mybir.dt.float32
BF16 = mybir.dt.bfloat16
I16 = mybir.dt.int16
AF = mybir.ActivationFunctionType
OP = mybir.AluOpType

N_CORES = 8
N_LR, N_HR = 25000, 100000
F_IN, F_W = 256, 512
LN_EPS = 1e-5
LAMBDA = 1.0507009873554804934193349852946
ALPHA = 1.6732632423543772848170429916717
LA = LAMBDA * ALPHA

NODES_PC = 12544          # 98 * 128, pad of 12500
N_SUB = NODES_PC // 128   # 98 sub-chunks of 128 nodes


def _tile_plan(nodes_pc):
    """List of (tile_start_sub, nsub) with nsub in {4, 2, 1}."""
    nsub_total = nodes_pc // 128
    plan = []
    s = 0
    while s < nsub_total:
        n = min(4, nsub_total - s)
        if n == 3:
            n = 2
        plan.append((s, n))
        s += n
    return plan


def build_nc(nodes_pc=NODES_PC, n_lr=N_LR):
    nsub_total = nodes_pc // 128
    nc = bacc.Bacc("TRN2", target_bir_lowering=False, debug=False)

    # ---- DRAM I/O ----
    v_d = nc.dram_tensor("v", [n_lr, F_IN], F32, kind="ExternalInput")
    erel_d = nc.dram_tensor("erel_t", [128, nsub_total], F32, kind="ExternalInput")
    idx_d = nc.dram_tensor("idx16", [128, nodes_pc // 16], I16, kind="ExternalInput")
    vskip_d = nc.dram_tensor("vskip_t", [128, nsub_total * F_IN], F32,
                             kind="ExternalInput")
    wv_d = nc.dram_tensor("wv", [128, 2048], BF16, kind="ExternalInput")
    w1t_d = nc.dram_tensor("w1t", [128, 2048], BF16, kind="ExternalInput")
    w2t_d = nc.dram_tensor("w2t", [128, 2048], BF16, kind="ExternalInput")
    w3t_d = nc.dram_tensor("w3t", [128, 2048], BF16, kind="ExternalInput")
    p4_d = nc.dram_tensor("p4", [4, 512], BF16, kind="ExternalInput")
    b3row_d = nc.dram_tensor("b3row", [1, 512], BF16, kind="ExternalInput")
    ones1_d = nc.dram_tensor("ones1", [1, 128], BF16, kind="ExternalInput")
    eyeb_d = nc.dram_tensor("eye_b", [128, 128], BF16, kind="ExternalInput")
    eyef_d = nc.dram_tensor("eye_f", [128, 128], F32, kind="ExternalInput")
    bias_d = nc.dram_tensor("biases", [128, 16], F32, kind="ExternalInput")
    # biases cols: 0-3 exp-bias L1 (b1p+ln A), 4-7 relu-bias L1 (b1p),
    #              8-11 exp-bias L2, 12-15 relu-bias L2
    cst_d = nc.dram_tensor("csts", [128, 8], F32, kind="ExternalInput")
    # csts cols: 0 -A1/1024, 1 A0/1024, 2 B2/1024, 3 -2*B1/1024, 4 B0/1024
    out_d = nc.dram_tensor("out", [nodes_pc, F_W], F32, kind="ExternalOutput")

    with tile.TileContext(nc) as tc:
        with (
            tc.tile_pool(name="const", bufs=1) as cp,
            tc.tile_pool(name="vin", bufs=3) as vinp,
            tc.tile_pool(name="vsc", bufs=3) as vscp,
            tc.tile_pool(name="stats", bufs=3) as stp,
            tc.tile_pool(name="xt", bufs=8) as xtp,
            tc.tile_pool(name="h1p", bufs=8) as h1p,
            tc.tile_pool(name="h2p", bufs=8) as h2p,
            tc.tile_pool(name="h3p", bufs=8) as h3p,
            tc.tile_pool(name="er", bufs=6) as erp,
            tc.tile_pool(name="fin", bufs=4) as finp,
            tc.tile_pool(name="xt_ps", bufs=2, space="PSUM") as xtps,
            tc.tile_pool(name="st_ps", bufs=2, space="PSUM") as stps,
            tc.tile_pool(name="z_ps", bufs=4, space="PSUM") as zps,
        ):
            nc.gpsimd.load_library(library_config.mlp)

            # ---- static loads ----
            def cload(dram, shape, dt):
                t = cp.tile(shape, dt)
                nc.sync.dma_start(t[:], dram.ap())
                return t

            wv = cload(wv_d, [128, 2048], BF16)
            w1t = cload(w1t_d, [128, 2048], BF16)
            w2t = cload(w2t_d, [128, 2048], BF16)
            w3t = cload(w3t_d, [128, 2048], BF16)
            p4 = cload(p4_d, [4, 512], BF16)
            b3row = cload(b3row_d, [1, 512], BF16)
            ones1 = cload(ones1_d, [1, 128], BF16)
            eye_b = cload(eyeb_d, [128, 128], BF16)
            eye_f = cload(eyef_d, [128, 128], F32)
            biases = cload(bias_d, [128, 16], F32)
            csts = cload(cst_d, [128, 8], F32)
            idx_sb = cload(idx_d, [128, nodes_pc // 16], I16)
            e_sb = cload(erel_d, [128, nsub_total], F32)

            vskip_ap = vskip_d.ap()

            for (s0, nsub) in _tile_plan(nodes_pc):
                nt = nsub * 128  # nodes in this tile

                # ---- load: gather v rows + v_skip slab ----
                vg = vinp.tile([128, nsub, F_IN], F32, tag="vg")
                nc.gpsimd.dma_gather(vg[:], v_d.ap(), idx_sb[:, s0 * 8:(s0 + nsub) * 8],
                                     nt, nt, F_IN)
                vs = vinp.tile([128, nsub, F_IN], F32, tag="vs")
                nc.sync.dma_start(vs[:], vskip_ap[:, s0 * F_IN:(s0 + nsub) * F_IN])

                # ---- LayerNorm stats ----
                sraw = stp.tile([128, 8, 6], F32, tag="sraw")
                nc.vector.bn_stats(sraw[:, 0:nsub, :], vg[:])
                nc.vector.bn_stats(sraw[:, 4:4 + nsub, :], vs[:])
                aggr = stp.tile([128, 4, 2], F32, tag="aggr")
                sr4 = sraw[:].rearrange("p (a c x) -> p a c x", a=2, c=4)
                for c in range(nsub):
                    nc.vector.bn_aggr(aggr[:, c, :], sr4[:, :, c, :])
                mean_v = aggr[:, 0:nsub, 0]
                var_v = aggr[:, 0:nsub, 1]
                r = e_sb[:, s0:s0 + nsub]

                st = stp.tile([128, 8, 4], F32, tag="st")  # temps, 4-wide each
                t1, r2, t2, t3, m2, t5, msq, muv = (st[:, i, 0:nsub] for i in range(8))
                st2 = stp.tile([128, 4, 4], F32, tag="st2")
                mu2, varv, lv = (st2[:, i, 0:nsub] for i in range(3))
                # mu = (-A1*r + A0 + 512*mean_v)/1024
                nc.vector.tensor_scalar(t1, r, csts[:, 0:1], csts[:, 1:2],
                                        OP.mult, OP.add)
                nc.vector.scalar_tensor_tensor(muv, mean_v, 0.5, t1, OP.mult, OP.add)
                # msq = (B2 r^2 - 2 B1 r + B0)/1024 + (var_v + mean_v^2)/2
                nc.vector.tensor_mul(r2, r, r)
                nc.vector.tensor_scalar(t2, r2, csts[:, 2:3], csts[:, 4:5],
                                        OP.mult, OP.add)
                nc.vector.scalar_tensor_tensor(t3, r, csts[:, 3:4], t2,
                                               OP.mult, OP.add)
                nc.vector.tensor_mul(m2, mean_v, mean_v)
                nc.vector.tensor_add(t5, m2, var_v)
                nc.vector.scalar_tensor_tensor(msq, t5, 0.5, t3, OP.mult, OP.add)
                # var = msq - mu^2 ; rstd = exp(-0.5*ln(var+eps))
                nc.vector.tensor_mul(mu2, muv, muv)
                nc.vector.tensor_sub(varv, msq, mu2)
                nc.scalar.activation(lv, varv, AF.Ln, bias=LN_EPS)
                s4 = stp.tile([128, nsub, 4], F32, tag="s4")  # [-r*rstd, rstd, -mu*rstd, 1]
                rstd = s4[:, :, 1]
                nc.scalar.activation(rstd, lv, AF.Exp, bias=0.0, scale=-0.5)
                nc.vector.scalar_tensor_tensor(s4[:, :, 0], r, -1.0, rstd,
                                               OP.mult, OP.mult)
                nc.vector.scalar_tensor_tensor(s4[:, :, 2], muv, -1.0, rstd,
                                               OP.mult, OP.mult)
                nc.gpsimd.memset(s4[:, :, 3], 1.0)

                # stats rows -> [4, nt] bf16 (transpose per sub-chunk)
                rhs4 = stp.tile([4, 512], BF16, tag="rhs4")
                for c in range(nsub):
                    sps = stps.tile([4, 128], F32, tag="sps")
                    nc.tensor.transpose(sps[:], s4[:, c, :], eye_f[:])
                    nc.vector.tensor_copy(rhs4[:, c * 128:(c + 1) * 128], sps[:])

                # ---- scale gathered features by rstd (per node) -> bf16 ----
                vgs = vscp.tile([128, nsub, F_IN], BF16, tag="vgs")
                vss = vscp.tile([128, nsub, F_IN], BF16, tag="vss")
                for c in range(nsub):
                    rs = s4[:, c, 1:2]
                    nc.gpsimd.tensor_scalar(vgs[:, c, :], vg[:, c, :], rs, None,
                                            OP.mult)
                    nc.gpsimd.tensor_scalar(vss[:, c, :], vs[:, c, :], rs, None,
                                            OP.mult)

                # ---- transpose to [feat, node] ----
                xts = []
                for kc in range(4):
                    src = vgs if kc < 2 else vss
                    off = (kc % 2) * 128
                    xps = xtps.tile([128, 512], F32, tag="xps")
                    for c in range(nsub):
                        nc.tensor.transpose(xps[:, c * 128:(c + 1) * 128],
                                            src[:, c, off:off + 128], eye_b[:])
                    xt = xtp.tile([128, 512], BF16, tag=f"xt{kc}")
                    nc.scalar.copy(xt[:, 0:nt], xps[:, 0:nt])
                    xts.append(xt)

                # ---- layer 0 (K = 4x128 features + 4 stats rows) ----
                h1s = []
                for mc in range(4):
                    z = zps.tile([128, 512], F32, tag="z")
                    for kc in range(4):
                        nc.tensor.matmul(z[:, 0:nt],
                                         wv[:, (kc * 4 + mc) * 128:(kc * 4 + mc + 1) * 128],
                                         xts[kc][:, 0:nt],
                                         start=(kc == 0), stop=False)
                    nc.tensor.matmul(z[:, 0:nt], p4[:, mc * 128:(mc + 1) * 128],
                                     rhs4[:, 0:nt], start=False, stop=True)
                    # selu (lambda/alpha folded): h = relu(z) + min(A*e^z, A)
                    e = erp.tile([128, 512], BF16, tag="e0")
                    nc.scalar.activation(e[:, 0:nt], z[:, 0:nt], AF.Exp,
                                         bias=math.log(ALPHA))
                    rl = erp.tile([128, 512], BF16, tag="r0")
                    nc.vector.tensor_scalar(rl[:, 0:nt], z[:, 0:nt], 0.0, None, OP.max)
                    h = h1p.tile([128, 512], BF16, tag=f"h1_{mc}")
                    nc.vector.scalar_tensor_tensor(h[:, 0:nt], e[:, 0:nt], ALPHA,
                                                   rl[:, 0:nt], OP.min, OP.add)
                    h1s.append(h)

                # ---- layer 1 ----
                h2s = []
                for mc in range(4):
                    z = zps.tile([128, 512], F32, tag="z")
                    for kc in range(4):
                        nc.tensor.matmul(z[:, 0:nt],
                                         w1t[:, (kc * 4 + mc) * 128:(kc * 4 + mc + 1) * 128],
                                         h1s[kc][:, 0:nt],
                                         start=(kc == 0), stop=(kc == 3))
                    e = erp.tile([128, 512], BF16, tag="e1")
                    nc.scalar.activation(e[:, 0:nt], z[:, 0:nt], AF.Exp,
                                         bias=biases[:, mc:mc + 1])
                    rl = erp.tile([128, 512], BF16, tag="r1")
                    nc.vector.tensor_scalar(rl[:, 0:nt], z[:, 0:nt],
                                            biases[:, 4 + mc:5 + mc], 0.0,
                                            OP.add, OP.max)
                    h = h2p.tile([128, 512], BF16, tag=f"h2_{mc}")
                    nc.vector.scalar_tensor_tensor(h[:, 0:nt], e[:, 0:nt], ALPHA,
                                                   rl[:, 0:nt], OP.min, OP.add)
                    h2s.append(h)

                # ---- layer 2 ----
                h3s = []
                for mc in range(4):
                    z = zps.tile([128, 512], F32, tag="z")
                    for kc in range(4):
                        nc.tensor.matmul(z[:, 0:nt],
                                         w2t[:, (kc * 4 + mc) * 128:(kc * 4 + mc + 1) * 128],
                                         h2s[kc][:, 0:nt],
                                         start=(kc == 0), stop=(kc == 3))
                    e = erp.tile([128, 512], BF16, tag="e2")
                    nc.scalar.activation(e[:, 0:nt], z[:, 0:nt], AF.Exp,
                                         bias=biases[:, 8 + mc:9 + mc])
                    rl = erp.tile([128, 512], BF16, tag="r2")
                    nc.scalar.activation(rl[:, 0:nt], z[:, 0:nt], AF.Relu,
                                         bias=biases[:, 12 + mc:13 + mc])
                    h = h3p.tile([128, 512], BF16, tag=f"h3_{mc}")
                    nc.gpsimd.scalar_tensor_tensor(h[:, 0:nt], e[:, 0:nt], ALPHA,
                                                   rl[:, 0:nt], OP.min, OP.add)
                    h3s.append(h)

                # ---- layer 3: stationary = h3 -> output is [node, feat] ----
                for c in range(nsub):
                    z = zps.tile([128, 512], F32, tag="z")
                    for kc in range(4):
                        nc.tensor.matmul(z[:], h3s[kc][:, c * 128:(c + 1) * 128],
                                         w3t[:, kc * 512:(kc + 1) * 512],
                                         start=(kc == 0), stop=False)
                    nc.tensor.matmul(z[:], ones1[0:1, :], b3row[0:1, :],
                                     start=False, stop=True)
                    e = finp.tile([128, 512], F32, tag="e3")
                    nc.scalar.activation(e[:], z[:], AF.Exp, bias=math.log(LA))
                    rl = finp.tile([128, 512], F32, tag="r3")
                    nc.scalar.activation(rl[:], z[:], AF.Relu, bias=0.0, scale=LAMBDA)
                    s3 = finp.tile([128, 512], F32, tag="s3")
                    nc.vector.scalar_tensor_tensor(s3[:], e[:], LA, rl[:],
                                                   OP.min, OP.add)
                    ob = finp.tile([128, 512], F32, tag="ob")
                    nc.vector.tensor_scalar(ob[:], s3[:], -LA, None, OP.add)
                    row0 = (s0 + c) * 128
                    nc.sync.dma_start(out_d.ap()[row0:row0 + 128, :], ob[:])

    nc.compile()
    return nc


def prep_host(v, e_rel, idx_hr_to_lr, v_skip, We, be, ln_g, ln_b,
              W0, b0, W1, b1, W2, b2, W3, b3, nodes_pc=NODES_PC):
    """Returns (shared_inputs, per_core_fn)."""
    f64 = np.float64
    bf = ml_dtypes.bfloat16
    W0g = (W0.astype(f64) * ln_g.astype(f64)[None, :])
    WeC = We.astype(f64)[:, 0]
    beC = be.astype(f64)

    def blocks(M):  # [512,512] -> [128, 2048] with block b = kc*4+mc
        out = np.empty((128, 2048), f64)
        MT = M.T  # [in, out]
        for kc in range(4):
            for mc in range(4):
                b = kc * 4 + mc
                out[:, b * 128:(b + 1) * 128] = MT[kc * 128:(kc + 1) * 128,
                                                   mc * 128:(mc + 1) * 128]
        return out

    wv = blocks(W0g[:, 512:1024])
    w1t = blocks(LAMBDA * W1.astype(f64))
    w2t = blocks(LAMBDA * W2.astype(f64))
    w3tT = (LAMBDA * W3.astype(f64)).T  # [in, out]
    w3t = np.concatenate([w3tT[k * 128:(k + 1) * 128, :] for k in range(4)], axis=1)

    p1 = W0g[:, :512] @ WeC
    p0 = W0g[:, :512] @ beC
    q = W0g.sum(1)
    c0 = b0.astype(f64) + W0.astype(f64) @ ln_b.astype(f64)
    p4 = np.stack([p1, p0, q, c0])  # [4, 512]

    b1p = b1.astype(f64) - LA * W1.astype(f64).sum(1)
    b2p = b2.astype(f64) - LA * W2.astype(f64).sum(1)
    b3p = b3.astype(f64) - LA * W3.astype(f64).sum(1)
    biases = np.stack([b1p + math.log(ALPHA), b1p,
                       b2p + math.log(ALPHA), b2p], axis=0)  # [4,512]
    biases = biases.reshape(4, 4, 128).transpose(2, 0, 1).reshape(128, 16)
    # col layout: [exp1 x4, relu1 x4, exp2 x4, relu2 x4] -- wait, need cols
    # 0-3 exp1(mc), 4-7 relu1(mc): reorder below
    # biases currently [128, (kind 4, mc 4)] with kind-major -> correct:
    # kind 0 = exp1 at cols 0-3, kind 1 = relu1 at 4-7, etc.

    A1, A0 = WeC.sum(), beC.sum()
    B2, B1, B0 = (WeC ** 2).sum(), (WeC * beC).sum(), (beC ** 2).sum()
    csts = np.zeros((128, 8), f64)
    csts[:, 0] = -A1 / 1024.0
    csts[:, 1] = A0 / 1024.0
    csts[:, 2] = B2 / 1024.0
    csts[:, 3] = -2.0 * B1 / 1024.0
    csts[:, 4] = B0 / 1024.0

    shared = dict(
        v=np.ascontiguousarray(v, np.float32),
        wv=wv.astype(bf), w1t=w1t.astype(bf), w2t=w2t.astype(bf),
        w3t=w3t.astype(bf), p4=p4.astype(bf),
        b3row=b3p[None, :].astype(bf),
        ones1=np.ones((1, 128), bf),
        eye_b=np.eye(128, dtype=np.float32).astype(bf),
        eye_f=np.eye(128, dtype=np.float32),
        biases=biases.astype(np.float32),
        csts=csts.astype(np.float32),
    )

    def per_core(core):
        lo = core * (N_HR // N_CORES)
        hi = lo + (N_HR // N_CORES)
        npc = nodes_pc
        er = np.zeros(npc, np.float32)
        er[:hi - lo] = e_rel[lo:hi, 0]
        ix = np.zeros(npc, np.int16)
        ix[:hi - lo] = idx_hr_to_lr[lo:hi].astype(np.int16)
        vsk = np.zeros((npc, F_IN), np.float32)
        vsk[:hi - lo] = v_skip[lo:hi]
        m = dict(shared)
        m["erel_t"] = np.ascontiguousarray(er.reshape(npc // 128, 128).T)
        m["idx16"] = np.ascontiguousarray(np.tile(ix.reshape(npc // 16, 16).T, (8, 1)))
        m["vskip_t"] = np.ascontiguousarray(
            vsk.reshape(npc // 128, 128, F_IN).transpose(1, 0, 2).reshape(128, -1))
        return m

    return shared, per_core


_NC_CACHE = {}


def kernel(v, e_rel, idx_hr_to_lr, v_skip, We, be, ln_g, ln_b,
           W0, b0, W1, b1, W2, b2, W3, b3):
    key = ("full", NODES_PC)
    if key not in _NC_CACHE:
        _NC_CACHE[key] = build_nc()
    nc = _NC_CACHE[key]
    _, per_core = prep_host(v, e_rel, idx_hr_to_lr, v_skip, We, be, ln_g, ln_b,
                            W0, b0, W1, b1, W2, b2, W3, b3)
    in_maps = [per_core(c) for c in range(N_CORES)]
    res = run_bass_kernel_spmd(nc, in_maps, core_ids=list(range(N_CORES)))
    per = N_HR // N_CORES
    out = np.concatenate([res.results[c]["out"][:per] for c in range(N_CORES)], axis=0)
    return out.astype(np.float32)


# revision 19
# speedup vs baseline: 1.6807x; 1.6807x over previous
"""Trainium2 Bass kernel for MeshUp message-passing block (gnn_message_passing).

reference math:
    e_enc = (-e_rel) @ We.T + be                      # [N,512]
    msg   = concat([e_enc, v[idx], v_skip], -1)       # [N,1024]
    h     = LayerNorm(msg) * g + b
    h     = selu(h @ W0.T + b0); selu(h @ W1.T + b1); selu(h @ W2.T + b2);
    out   = selu(h @ W3.T + b3)                       # [N,512]

Strategy (per core; N_hr sharded 8 ways, v table replicated per core):
  - activations live as [feat, node] tiles (feat chunks of 128 on partitions,
    nodes on the free dim) -> layers 1-2 need no transposes at all.
  - the e_enc half of msg plus the whole LayerNorm affine plus b0 fold into
    layer 0's matmul as a rank-4 (K=4) correction:
        z0 = W0g_v @ (v_cat * rstd)  +  [p1;p0;q;c0]^T @ [-r*rstd; rstd; -mu*rstd; 1]
    so e_enc is never materialized and LN costs only a stats pass (bn_stats
    on the gathered features + closed-form quadratic for the e_enc half).
  - selu(z) = L*relu(z) + min(L*A*exp(z), L*A) - L*A with the -L*A and L
    folded into the next layer's weights/biases -> 1 ACT exp pass, 1 relu
    pass, 1 fused scalar_tensor_tensor per element.
  - layer 3 uses h3 as the matmul stationary operand so its output lands
    [node, feat] and DMAs straight to DRAM.
  - matmuls in fp16 (1 cyc/row on PE like bf16, but 10 mantissa bits),
    fp32 PSUM accumulate; stats + final output in fp32.
  - elementwise sites operate on [128, 1024] pairs (two PSUM banks) to
    amortize per-op overhead; GPSIMD does only the gather (its per-op
    dispatch overhead under Tile is ~5us, useless for elementwise).
"""

import math
import os
import sys

for _p in ("/opt/trn_rl_repo", "/root/.axon_site/_ro/trn_rl_repo"):
    if os.path.isdir(_p) and _p not in sys.path:
        sys.path.insert(0, _p)

import numpy as np
import ml_dtypes  # noqa: F401

import concourse.bacc as bacc
import concourse.tile as tile
from concourse import library_config, mybir
from concourse import hw_specs
from concourse.bass_utils import run_bass_kernel_spmd

F32 = mybir.dt.float32
F16 = mybir.dt.float16
I16 = mybir.dt.int16
AF = mybir.ActivationFunctionType
OP = mybir.AluOpType
AX = mybir.AxisListType

N_CORES = 8
N_LR, N_HR = 25000, 100000
F_IN, F_W = 256, 512
LN_EPS = 1e-5
LAMBDA = 1.0507009873554804934193349852946
ALPHA = 1.6732632423543772848170429916717
LA = LAMBDA * ALPHA

NODES_PC = 12544          # 98 * 128, pad of 12500

_orig_get_tables = hw_specs.get_activation_tables
_patched = False


def _patch_act_tables():
    """Make Exp and Ln resolve to the one set containing both, so the
    per-tile Ln/Exp mix doesn't thrash ACT_TABLE_LOAD (2.7us each)."""
    global _patched
    if _patched:
        return
    _patched = True

    def patched(arch):
        t = dict(_orig_get_tables(arch))
        combined = None
        for name, fns in t.items():
            if AF.Exp in fns and AF.Ln in fns:
                combined = name
                break
        if combined:
            for name in t:
                if name != combined:
                    t[name] = t[name] - {AF.Exp, AF.Ln}
        return t

    bacc.get_activation_tables = patched


def _tile_plan(nodes_pc):
    """List of (tile_start_sub, nsub) with nsub in {4, 2}."""
    nsub_total = nodes_pc // 128
    assert nsub_total % 2 == 0
    plan = []
    s = 0
    while s < nsub_total:
        n = min(4, nsub_total - s)
        if n == 3:
            n = 2
        plan.append((s, n))
        s += n
    return plan


def build_nc(nodes_pc=NODES_PC, n_lr=N_LR):
    _patch_act_tables()
    nsub_total = nodes_pc // 128
    nc = bacc.Bacc("TRN2", target_bir_lowering=False, debug=False)

    # ---- DRAM I/O ----
    v_d = nc.dram_tensor("v", [n_lr, F_IN], F32, kind="ExternalInput")
    erel_d = nc.dram_tensor("erel_t", [128, nsub_total], F32, kind="ExternalInput")
    idx_d = nc.dram_tensor("idx16", [128, nodes_pc // 16], I16, kind="ExternalInput")
    vskip_d = nc.dram_tensor("vskip_t", [128, nsub_total * F_IN], F32,
                             kind="ExternalInput")
    wv_d = nc.dram_tensor("wv", [128, 2048], F16, kind="ExternalInput")
    w1t_d = nc.dram_tensor("w1t", [128, 2048], F16, kind="ExternalInput")
    w2t_d = nc.dram_tensor("w2t", [128, 2048], F16, kind="ExternalInput")
    w3t_d = nc.dram_tensor("w3t", [128, 2048], F16, kind="ExternalInput")
    p4_d = nc.dram_tensor("p4", [4, 512], F16, kind="ExternalInput")
    b3row_d = nc.dram_tensor("b3row", [1, 512], F16, kind="ExternalInput")
    ones1_d = nc.dram_tensor("ones1", [1, 128], F16, kind="ExternalInput")
    eyeb_d = nc.dram_tensor("eye_b", [128, 128], F16, kind="ExternalInput")
    eyef_d = nc.dram_tensor("eye_f", [128, 128], F32, kind="ExternalInput")
    bias_d = nc.dram_tensor("biases", [128, 16], F32, kind="ExternalInput")
    # biases cols: 0-3 exp-bias L1 (b1p+ln A), 4-7 relu-bias L1 (b1p),
    #              8-11 exp-bias L2, 12-15 relu-bias L2
    cst_d = nc.dram_tensor("csts", [128, 12], F32, kind="ExternalInput")
    # csts cols: 0 -A1/1024, 1 A0/1024, 2 B2/1024, 3 -2*B1/1024, 4 B0/1024,
    #            5 LN_EPS, 6 ln(ALPHA), 7 ln(LA)
    out_d = nc.dram_tensor("out", [nodes_pc, F_W], F32, kind="ExternalOutput")

    with tile.TileContext(nc) as tc:
        with (
            tc.tile_pool(name="const", bufs=1) as cp,
            tc.tile_pool(name="vin", bufs=3) as vinp,
            tc.tile_pool(name="vsc", bufs=2) as vscp,
            tc.tile_pool(name="stats", bufs=2) as stp,
            tc.tile_pool(name="xt", bufs=2) as xtp,
            tc.tile_pool(name="h1p", bufs=2) as h1p,
            tc.tile_pool(name="h2p", bufs=2) as h2p,
            tc.tile_pool(name="h3p", bufs=2) as h3p,
            tc.tile_pool(name="er", bufs=2) as erp,
            tc.tile_pool(name="fin", bufs=2) as finp,
            tc.tile_pool(name="xt_ps", bufs=2, space="PSUM") as xtps,
            tc.tile_pool(name="z_ps", bufs=3, space="PSUM") as zps,
        ):
            nc.gpsimd.load_library(library_config.mlp)

            # ---- static loads ----
            def cload(dram, shape, dt, tag):
                t = cp.tile(shape, dt, tag=tag)
                nc.sync.dma_start(t[:], dram.ap())
                return t

            wv = cload(wv_d, [128, 2048], F16, "c_wv")
            w1t = cload(w1t_d, [128, 2048], F16, "c_w1t")
            w2t = cload(w2t_d, [128, 2048], F16, "c_w2t")
            w3t = cload(w3t_d, [128, 2048], F16, "c_w3t")
            p4 = cload(p4_d, [4, 512], F16, "c_p4")
            b3row = cload(b3row_d, [1, 512], F16, "c_b3row")
            ones1 = cload(ones1_d, [1, 128], F16, "c_ones1")
            eye_b = cload(eyeb_d, [128, 128], F16, "c_eyeb")
            eye_f = cload(eyef_d, [128, 128], F32, "c_eyef")
            biases = cload(bias_d, [128, 16], F32, "c_biases")
            csts = cload(cst_d, [128, 12], F32, "c_csts")
            idx_sb = cload(idx_d, [128, nodes_pc // 16], I16, "c_idx")
            e_sb = cload(erel_d, [128, nsub_total], F32, "c_erel")

            vskip_ap = vskip_d.ap()

            for (s0, nsub) in _tile_plan(nodes_pc):
                nt = nsub * 128   # nodes in this tile
                nh = nsub // 2    # number of mc/sub-chunk PAIRS

                # ---- load: gather v rows + v_skip slab ----
                vg = vinp.tile([128, nsub, F_IN], F32, tag="vg")
                nc.gpsimd.dma_gather(vg[:], v_d.ap(),
                                     idx_sb[:, s0 * 8:(s0 + nsub) * 8],
                                     nt, nt, F_IN)
                vs = vinp.tile([128, nsub, F_IN], F32, tag="vs")
                nc.sync.dma_start(vs[:], vskip_ap[:, s0 * F_IN:(s0 + nsub) * F_IN])

                # ---- LayerNorm stats ----  (high priority: this chain
                # gates the next tile's PE work, so run it as soon as the
                # gather lands rather than behind the current tile's selu)
                hp = tc.high_priority()
                hp.__enter__()
                # S[:, a, c, g, 0:3] : a in {vg, vs}, c = sub-chunk,
                # g in {even, odd}; triple = (count, mean, count*var)
                S = stp.tile([128, 2, 4, 6], F32, tag="S")
                bns = []
                for c in range(nsub):
                    # HW BNStats handles exactly one 6-wide group per op
                    bns.append(nc.vector.bn_stats(S[:, 0, c, :], vg[:, c, :]))
                    bns.append(nc.vector.bn_stats(S[:, 1, c, :], vs[:, c, :]))
                # per sub-chunk: sum of 4 group means / count*vars over (a,g)
                red = stp.tile([128, 4, 4], F32, tag="red")  # rows: Sm, Sv, Sm2, -
                Sm = red[:, 0, 0:nsub]
                Sv = red[:, 1, 0:nsub]
                Sm2 = red[:, 2, 0:nsub]
                msc = stp.tile([128, 4, 2, 2], F32, tag="msc")  # squared means
                # view with (c) outer and (a,g) innermost to reduce over:
                m_view = S[:].rearrange("p a c (g x) -> p c a g x", g=2)
                _rr = [
                    nc.vector.tensor_reduce(Sm, m_view[:, 0:nsub, :, :, 1],
                                            AX.XY, OP.add),
                    nc.vector.tensor_reduce(Sv, m_view[:, 0:nsub, :, :, 2],
                                            AX.XY, OP.add),
                    nc.vector.tensor_tensor(msc[:, 0:nsub, :, :],
                                            m_view[:, 0:nsub, :, :, 1],
                                            m_view[:, 0:nsub, :, :, 1],
                                            OP.mult),
                ]
                _rr.append(nc.vector.tensor_reduce(Sm2, msc[:, 0:nsub, :, :],
                                                   AX.XY, OP.add))
                for _r in _rr:
                    for _b in bns:
                        tile.add_dep_helper(_r.ins, _b.ins, sync=True,
                                            reason="bn_stats -> strided reduce")

                r = e_sb[:, s0:s0 + nsub]
                st = stp.tile([128, 8, 4], F32, tag="st")
                t1, r2, t2, t3, t4, msq, muv, mu2 = (st[:, i, 0:nsub]
                                                     for i in range(8))
                st2 = stp.tile([128, 2, 4], F32, tag="st2")
                varv, lv = (st2[:, i, 0:nsub] for i in range(2))
                # mu = (-A1*r + A0)/1024 + Sm/8      (Sm = sum of 4 means)
                nc.vector.tensor_scalar(t1, r, csts[:, 0:1], csts[:, 1:2],
                                        OP.mult, OP.add)
                nc.vector.scalar_tensor_tensor(muv, Sm, 0.125, t1, OP.mult, OP.add)
                # msq = (B2 r^2 - 2 B1 r + B0)/1024 + Sv/1024 + Sm2/8
                nc.vector.tensor_mul(r2, r, r)
                nc.vector.tensor_scalar(t2, r2, csts[:, 2:3], csts[:, 4:5],
                                        OP.mult, OP.add)
                nc.vector.scalar_tensor_tensor(t3, r, csts[:, 3:4], t2,
                                               OP.mult, OP.add)
                nc.vector.scalar_tensor_tensor(t4, Sv, 1.0 / 128.0, Sm2,
                                               OP.mult, OP.add)
                nc.vector.scalar_tensor_tensor(msq, t4, 0.125, t3, OP.mult, OP.add)
                nc.vector.tensor_mul(mu2, muv, muv)
                nc.vector.tensor_sub(varv, msq, mu2)
                nc.scalar.activation(lv, varv, AF.Ln, bias=csts[:, 5:6])
                s4 = stp.tile([128, nsub, 4], F32, tag="s4")
                rstd = s4[:, :, 1]
                nc.scalar.activation(rstd, lv, AF.Exp, bias=0.0, scale=-0.5)
                nc.vector.scalar_tensor_tensor(s4[:, :, 0], r, -1.0, rstd,
                                               OP.mult, OP.mult)
                nc.vector.scalar_tensor_tensor(s4[:, :, 2], muv, -1.0, rstd,
                                               OP.mult, OP.mult)
                nc.vector.tensor_scalar(s4[:, :, 3], rstd, 0.0, 1.0,
                                        OP.mult, OP.add)

                # stats rows -> [4, nt] fp16 (transpose per sub-chunk)
                rhs4 = stp.tile([4, 512], F16, tag="rhs4")
                for c in range(nsub):
                    sps = xtps.tile([4, 128], F32, tag="xps")
                    nc.tensor.transpose(sps[:], s4[:, c, :], eye_f[:])
                    nc.vector.tensor_copy(rhs4[:, c * 128:(c + 1) * 128], sps[:])

                # ---- scale gathered features by rstd (per node) -> fp16 ----
                vgs = vscp.tile([128, nsub, F_IN], F16, tag="vgs")
                vss = vscp.tile([128, nsub, F_IN], F16, tag="vss")
                for c in range(nsub):
                    rs = s4[:, c, 1:2]
                    nc.vector.tensor_scalar(vgs[:, c, :], vg[:, c, :], rs, None,
                                            OP.mult)
                    nc.vector.tensor_scalar(vss[:, c, :], vs[:, c, :], rs, None,
                                            OP.mult)

                # ---- transpose to [feat, node] ----
                xts = []
                for kc in range(4):
                    src = vgs if kc < 2 else vss
                    off = (kc % 2) * 128
                    xps = xtps.tile([128, 512], F16, tag="xps")
                    for c in range(nsub):
                        nc.tensor.transpose(xps[:, c * 128:(c + 1) * 128],
                                            src[:, c, off:off + 128], eye_b[:])
                    xt = xtp.tile([128, 512], F16, tag=f"xt{kc}")
                    if kc < 3:
                        nc.scalar.copy(xt[:, 0:nt], xps[:, 0:nt])
                    else:
                        nc.vector.tensor_copy(xt[:, 0:nt], xps[:, 0:nt])
                    xts.append(xt)
                hp.__exit__(None, None, None)

                # ---- layers 0-2: [feat, node]; mc pairs share a PSUM tile ----
                def ff_layer(wmat, hsrcs, hpool, htag, lix):
                    """z = W @ hsrc (+ stats rows for layer 0), selu -> h tiles.
                    Returns list of 2 tiles [128, 2, 512] (mc pairs)."""
                    houts = []
                    for half in range(2):
                        z = zps.tile([128, 2, 512], F32, tag="z")
                        for i in range(2):
                            mc = 2 * half + i
                            for kc in range(4):
                                b = kc * 4 + mc
                                nc.tensor.matmul(
                                    z[:, i, 0:nt],
                                    wmat[:, b * 128:(b + 1) * 128],
                                    hsrcs[kc][:, 0:nt],
                                    start=(kc == 0),
                                    stop=(lix != 0 and kc == 3))
                            if lix == 0:
                                nc.tensor.matmul(z[:, i, 0:nt],
                                                 p4[:, mc * 128:(mc + 1) * 128],
                                                 rhs4[:, 0:nt],
                                                 start=False, stop=True)
                        e = erp.tile([128, 2, 512], F16, tag=f"e{lix}")
                        rl = erp.tile([128, 2, 512], F16, tag=f"r{lix}")
                        fused = (nt == 512 and lix == 0)
                        if fused:
                            zv = z[:, :, 0:nt].rearrange("p a n -> p (a n)")
                            ev = e[:, :, 0:nt].rearrange("p a n -> p (a n)")
                            rv = rl[:, :, 0:nt].rearrange("p a n -> p (a n)")
                            nc.scalar.activation(ev, zv, AF.Exp, bias=csts[:, 6:7])
                            nc.vector.tensor_scalar(rv, zv, 0.0, None, OP.max)
                        else:
                            bcol = biases[:, 8 * (lix - 1):] if lix else None
                            for i in range(2):
                                mc = 2 * half + i
                                if lix == 0:
                                    nc.scalar.activation(
                                        e[:, i, 0:nt], z[:, i, 0:nt], AF.Exp,
                                        bias=csts[:, 6:7])
                                    nc.vector.tensor_scalar(
                                        rl[:, i, 0:nt], z[:, i, 0:nt], 0.0,
                                        None, OP.max)
                                    continue
                                nc.scalar.activation(
                                    e[:, i, 0:nt], z[:, i, 0:nt], AF.Exp,
                                    bias=bcol[:, mc:mc + 1])
                                nc.scalar.activation(
                                    rl[:, i, 0:nt], z[:, i, 0:nt], AF.Relu,
                                    bias=bcol[:, 4 + mc:5 + mc])
                        h = hpool.tile([128, 2, 512], F16, tag=f"{htag}_{half}")
                        if nt == 512:
                            hv = h[:].rearrange("p a n -> p (a n)")
                            ev = e[:].rearrange("p a n -> p (a n)")
                            rv = rl[:].rearrange("p a n -> p (a n)")
                            nc.vector.scalar_tensor_tensor(hv, ev, ALPHA, rv,
                                                           OP.min, OP.add)
                        else:
                            for i in range(2):
                                nc.vector.scalar_tensor_tensor(
                                    h[:, i, 0:nt], e[:, i, 0:nt], ALPHA,
                                    rl[:, i, 0:nt], OP.min, OP.add)
                        houts.append(h)
                    return houts

                def hslice(hts, kc):
                    return hts[kc // 2][:, kc % 2, :]

                h1s = ff_layer(wv, xts, h1p, "h1", 0)
                h1v = [hslice(h1s, kc) for kc in range(4)]
                h2s = ff_layer(w1t, h1v, h2p, "h2", 1)
                h2v = [hslice(h2s, kc) for kc in range(4)]
                h3s = ff_layer(w2t, h2v, h3p, "h3", 2)
                h3v = [hslice(h3s, kc) for kc in range(4)]

                # ---- layer 3: stationary = h3 -> output [node, feat] ----
                for half in range(nh):
                    z = zps.tile([128, 2, 512], F32, tag="z")
                    for i in range(2):
                        c = 2 * half + i
                        for kc in range(4):
                            nc.tensor.matmul(z[:, i, :],
                                             h3v[kc][:, c * 128:(c + 1) * 128],
                                             w3t[:, kc * 512:(kc + 1) * 512],
                                             start=(kc == 0), stop=False)
                        nc.tensor.matmul(z[:, i, :], ones1[0:1, :], b3row[0:1, :],
                                         start=False, stop=True)
                    zv = z[:].rearrange("p a n -> p (a n)")
                    e = finp.tile([128, 1024], F32, tag="e3")
                    nc.scalar.activation(e[:], zv, AF.Exp, bias=csts[:, 7:8])
                    rl = finp.tile([128, 1024], F32, tag="r3")
                    nc.scalar.activation(rl[:], zv, AF.Relu, bias=0.0, scale=LAMBDA)
                    s3 = finp.tile([128, 1024], F32, tag="s3")
                    nc.vector.scalar_tensor_tensor(s3[:], e[:], LA, rl[:],
                                                   OP.min, OP.add)
                    ob = finp.tile([128, 2, 512], F32, tag="ob")
                    nc.scalar.activation(
                        ob[:].rearrange("p a n -> p (a n)"), s3[:], AF.Identity,
                        bias=csts[:, 8:9])
                    row0 = (s0 + 2 * half) * 128
                    dst = out_d.ap()[row0:row0 + 256, :].rearrange(
                        "(a p) f -> p a f", p=128)
                    nc.sync.dma_start(dst, ob[:])

    nc.compile()
    return nc


def prep_host(v, e_rel, idx_hr_to_lr, v_skip, We, be, ln_g, ln_b,
              W0, b0, W1, b1, W2, b2, W3, b3, nodes_pc=NODES_PC, n_cores=N_CORES):
    """Returns (shared_inputs, per_core_fn)."""
    f64 = np.float64
    hdt = np.float16  # must match device matmul dtype F16
    W0g = (np.asarray(W0, f64) * np.asarray(ln_g, f64)[None, :])
    WeC = np.asarray(We, f64)[:, 0]
    beC = np.asarray(be, f64)

    def blocks(M):  # [512,512] -> [128, 2048] with block b = kc*4+mc
        out = np.empty((128, 2048), f64)
        MT = M.T  # [in, out]
        for kc in range(4):
            for mc in range(4):
                b = kc * 4 + mc
                out[:, b * 128:(b + 1) * 128] = MT[kc * 128:(kc + 1) * 128,
                                                   mc * 128:(mc + 1) * 128]
        return out

    wv = blocks(W0g[:, 512:1024])
    w1t = blocks(LAMBDA * np.asarray(W1, f64))
    w2t = blocks(LAMBDA * np.asarray(W2, f64))
    w3tT = (LAMBDA * np.asarray(W3, f64)).T  # [in, out]
    w3t = np.concatenate([w3tT[k * 128:(k + 1) * 128, :] for k in range(4)], axis=1)

    p1 = W0g[:, :512] @ WeC
    p0 = W0g[:, :512] @ beC
    q = W0g.sum(1)
    c0 = np.asarray(b0, f64) + np.asarray(W0, f64) @ np.asarray(ln_b, f64)
    p4 = np.stack([p1, p0, q, c0])  # [4, 512]

    b1p = np.asarray(b1, f64) - LA * np.asarray(W1, f64).sum(1)
    b2p = np.asarray(b2, f64) - LA * np.asarray(W2, f64).sum(1)
    b3p = np.asarray(b3, f64) - LA * np.asarray(W3, f64).sum(1)
    biases = np.stack([b1p + math.log(ALPHA), b1p,
                       b2p + math.log(ALPHA), b2p], axis=0)  # [4(kind), 512]
    # -> [128, 16] with col = kind*4 + mc, partition p holds b[mc*128+p]
    biases = biases.reshape(4, 4, 128).transpose(2, 0, 1).reshape(128, 16)

    A1, A0 = WeC.sum(), beC.sum()
    B2, B1, B0 = (WeC ** 2).sum(), (WeC * beC).sum(), (beC ** 2).sum()
    csts = np.zeros((128, 12), f64)
    csts[:, 0] = -A1 / 1024.0
    csts[:, 1] = A0 / 1024.0
    csts[:, 2] = B2 / 1024.0
    csts[:, 3] = -2.0 * B1 / 1024.0
    csts[:, 4] = B0 / 1024.0
    csts[:, 5] = LN_EPS
    csts[:, 6] = math.log(ALPHA)
    csts[:, 7] = math.log(LA)
    csts[:, 8] = -LA

    shared = dict(
        v=np.ascontiguousarray(np.asarray(v, np.float32)),
        wv=wv.astype(hdt), w1t=w1t.astype(hdt), w2t=w2t.astype(hdt),
        w3t=w3t.astype(hdt), p4=p4.astype(hdt),
        b3row=b3p[None, :].astype(hdt),
        ones1=np.ones((1, 128), hdt),
        eye_b=np.eye(128, dtype=np.float32).astype(hdt),
        eye_f=np.eye(128, dtype=np.float32),
        biases=biases.astype(np.float32),
        csts=csts.astype(np.float32),
    )

    e_rel = np.asarray(e_rel, np.float32)
    idx_hr_to_lr = np.asarray(idx_hr_to_lr)
    v_skip = np.asarray(v_skip, np.float32)
    n_hr = e_rel.shape[0]
    per = n_hr // n_cores

    def per_core(core):
        lo = core * per
        hi = lo + per
        npc = nodes_pc
        er = np.zeros(npc, np.float32)
        er[:hi - lo] = e_rel[lo:hi, 0]
        ix = np.zeros(npc, np.int16)
        ix[:hi - lo] = idx_hr_to_lr[lo:hi].astype(np.int16)
        vsk = np.zeros((npc, F_IN), np.float32)
        vsk[:hi - lo] = v_skip[lo:hi]
        m = dict(shared)
        m["erel_t"] = np.ascontiguousarray(er.reshape(npc // 128, 128).T)
        m["idx16"] = np.ascontiguousarray(np.tile(ix.reshape(npc // 16, 16).T, (8, 1)))
        m["vskip_t"] = np.ascontiguousarray(
            vsk.reshape(npc // 128, 128, F_IN).transpose(1, 0, 2).reshape(128, -1))
        return m

    return shared, per_core


_NC_CACHE = {}


def kernel(v, e_rel, idx_hr_to_lr, v_skip, We, be, ln_g, ln_b,
           W0, b0, W1, b1, W2, b2, W3, b3):
    key = ("full", NODES_PC)
    if key not in _NC_CACHE:
        _NC_CACHE[key] = build_nc()
    nc = _NC_CACHE[key]
    _, per_core = prep_host(v, e_rel, idx_hr_to_lr, v_skip, We, be, ln_g, ln_b,
                            W0, b0, W1, b1, W2, b2, W3, b3)
    in_maps = [per_core(c) for c in range(N_CORES)]
    res = run_bass_kernel_spmd(nc, in_maps, core_ids=list(range(N_CORES)))
    per = N_HR // N_CORES
    out = np.concatenate([res.results[c]["out"][:per] for c in range(N_CORES)], axis=0)
    return np.asarray(out, np.float32)


# revision 20
# speedup vs baseline: 2.0105x; 1.1962x over previous
"""Trainium2 Bass kernel for MeshUp message-passing block (gnn_message_passing).

reference math:
    e_enc = (-e_rel) @ We.T + be                      # [N,512]
    msg   = concat([e_enc, v[idx], v_skip], -1)       # [N,1024]
    h     = LayerNorm(msg) * g + b
    h     = selu(h @ W0.T + b0); selu(h @ W1.T + b1); selu(h @ W2.T + b2);
    out   = selu(h @ W3.T + b3)                       # [N,512]

Strategy (per core; N_hr sharded 8 ways, v table replicated per core):
  - activations live as [feat, node] tiles (feat chunks of 128 on partitions,
    nodes on the free dim) -> layers 1-2 need no transposes at all.
  - the e_enc half of msg plus the whole LayerNorm affine plus b0 fold into
    layer 0's matmul as a rank-4 (K=4) correction:
        z0 = W0g_v @ (v_cat * rstd)  +  [p1;p0;q;c0]^T @ [-r*rstd; rstd; -mu*rstd; 1]
    so e_enc is never materialized and LN costs only a stats pass (bn_stats
    on the gathered features + closed-form quadratic for the e_enc half).
  - selu(z) = L*relu(z) + min(L*A*exp(z), L*A) - L*A with the -L*A and L
    folded into the next layer's weights/biases -> 1 ACT exp pass, 1 relu
    pass, 1 fused scalar_tensor_tensor per element.
  - layer 3 uses h3 as the matmul stationary operand so its output lands
    [node, feat] and DMAs straight to DRAM.
  - matmuls in fp16 (1 cyc/row on PE like bf16, but 10 mantissa bits),
    fp32 PSUM accumulate; stats + final output in fp32.
  - elementwise sites operate on [128, 1024] pairs (two PSUM banks) to
    amortize per-op overhead; GPSIMD does only the gather (its per-op
    dispatch overhead under Tile is ~5us, useless for elementwise).
"""

import math
import os
import sys

for _p in ("/opt/trn_rl_repo", "/root/.axon_site/_ro/trn_rl_repo"):
    if os.path.isdir(_p) and _p not in sys.path:
        sys.path.insert(0, _p)

import numpy as np
import ml_dtypes  # noqa: F401

import concourse.bacc as bacc
import concourse.tile as tile
from concourse import library_config, mybir
from concourse import hw_specs
from concourse.bass_utils import run_bass_kernel_spmd

F32 = mybir.dt.float32
F16 = mybir.dt.float16
I16 = mybir.dt.int16
AF = mybir.ActivationFunctionType
OP = mybir.AluOpType
AX = mybir.AxisListType

N_CORES = 8
N_LR, N_HR = 25000, 100000
F_IN, F_W = 256, 512
LN_EPS = 1e-5
LAMBDA = 1.0507009873554804934193349852946
ALPHA = 1.6732632423543772848170429916717
LA = LAMBDA * ALPHA

NODES_PC = 12544          # 98 * 128, pad of 12500

_orig_get_tables = hw_specs.get_activation_tables
_patched = False


def _patch_act_tables():
    """Make Exp and Ln resolve to the one set containing both, so the
    per-tile Ln/Exp mix doesn't thrash ACT_TABLE_LOAD (2.7us each)."""
    global _patched
    if _patched:
        return
    _patched = True

    def patched(arch):
        t = dict(_orig_get_tables(arch))
        combined = None
        for name, fns in t.items():
            if AF.Exp in fns and AF.Ln in fns:
                combined = name
                break
        if combined:
            for name in t:
                if name != combined:
                    t[name] = t[name] - {AF.Exp, AF.Ln}
        return t

    bacc.get_activation_tables = patched


def _tile_plan(nodes_pc):
    """List of (tile_start_sub, nsub) with nsub in {4, 2}."""
    nsub_total = nodes_pc // 128
    assert nsub_total % 2 == 0
    plan = []
    s = 0
    while s < nsub_total:
        n = min(4, nsub_total - s)
        if n == 3:
            n = 2
        plan.append((s, n))
        s += n
    return plan


def build_nc(nodes_pc=NODES_PC, n_lr=N_LR):
    _patch_act_tables()
    nsub_total = nodes_pc // 128
    nc = bacc.Bacc("TRN2", target_bir_lowering=False, debug=False)

    # ---- DRAM I/O ----
    v_d = nc.dram_tensor("v", [n_lr, F_IN], F32, kind="ExternalInput")
    erel_d = nc.dram_tensor("erel_t", [128, nsub_total], F32, kind="ExternalInput")
    idx_d = nc.dram_tensor("idx16", [128, nodes_pc // 16], I16, kind="ExternalInput")
    vskip_d = nc.dram_tensor("vskip_t", [128, nsub_total * F_IN], F32,
                             kind="ExternalInput")
    wv_d = nc.dram_tensor("wv", [128, 2048], F16, kind="ExternalInput")
    w1t_d = nc.dram_tensor("w1t", [128, 2048], F16, kind="ExternalInput")
    w2t_d = nc.dram_tensor("w2t", [128, 2048], F16, kind="ExternalInput")
    w3t_d = nc.dram_tensor("w3t", [128, 2048], F16, kind="ExternalInput")
    p4_d = nc.dram_tensor("p4", [4, 512], F16, kind="ExternalInput")
    b3row_d = nc.dram_tensor("b3row", [1, 512], F16, kind="ExternalInput")
    ones1_d = nc.dram_tensor("ones1", [1, 128], F16, kind="ExternalInput")
    eyeb_d = nc.dram_tensor("eye_b", [128, 128], F16, kind="ExternalInput")
    eyef_d = nc.dram_tensor("eye_f", [128, 128], F32, kind="ExternalInput")
    bias_d = nc.dram_tensor("biases", [128, 16], F32, kind="ExternalInput")
    # biases cols: 0-3 exp-bias L1 (b1p+ln A), 4-7 relu-bias L1 (b1p),
    #              8-11 exp-bias L2, 12-15 relu-bias L2
    cst_d = nc.dram_tensor("csts", [128, 12], F32, kind="ExternalInput")
    # csts cols: 0 -A1/1024, 1 A0/1024, 2 B2/1024, 3 -2*B1/1024, 4 B0/1024,
    #            5 LN_EPS, 6 ln(ALPHA), 7 ln(LA)
    out_d = nc.dram_tensor("out", [nodes_pc, F_W], F32, kind="ExternalOutput")

    with tile.TileContext(nc) as tc:
        with (
            tc.tile_pool(name="const", bufs=1) as cp,
            tc.tile_pool(name="vin", bufs=4) as vinp,
            tc.tile_pool(name="vsc", bufs=3) as vscp,
            tc.tile_pool(name="stats", bufs=3) as stp,
            tc.tile_pool(name="xt", bufs=3) as xtp,
            tc.tile_pool(name="h1p", bufs=2) as h1p,
            tc.tile_pool(name="h2p", bufs=2) as h2p,
            tc.tile_pool(name="h3p", bufs=2) as h3p,
            tc.tile_pool(name="er", bufs=2) as erp,
            tc.tile_pool(name="fin", bufs=2) as finp,
            tc.tile_pool(name="xt_ps", bufs=2, space="PSUM") as xtps,
            tc.tile_pool(name="z_ps", bufs=3, space="PSUM") as zps,
        ):
            nc.gpsimd.load_library(library_config.mlp)

            # ---- static loads ----
            def cload(dram, shape, dt, tag):
                t = cp.tile(shape, dt, tag=tag)
                nc.sync.dma_start(t[:], dram.ap())
                return t

            wv = cload(wv_d, [128, 2048], F16, "c_wv")
            w1t = cload(w1t_d, [128, 2048], F16, "c_w1t")
            w2t = cload(w2t_d, [128, 2048], F16, "c_w2t")
            w3t = cload(w3t_d, [128, 2048], F16, "c_w3t")
            p4 = cload(p4_d, [4, 512], F16, "c_p4")
            b3row = cload(b3row_d, [1, 512], F16, "c_b3row")
            ones1 = cload(ones1_d, [1, 128], F16, "c_ones1")
            eye_b = cload(eyeb_d, [128, 128], F16, "c_eyeb")
            eye_f = cload(eyef_d, [128, 128], F32, "c_eyef")
            biases = cload(bias_d, [128, 16], F32, "c_biases")
            csts = cload(cst_d, [128, 12], F32, "c_csts")
            idx_sb = cload(idx_d, [128, nodes_pc // 16], I16, "c_idx")
            e_sb = cload(erel_d, [128, nsub_total], F32, "c_erel")

            vskip_ap = vskip_d.ap()

            for (s0, nsub) in _tile_plan(nodes_pc):
                nt = nsub * 128   # nodes in this tile
                nh = nsub // 2    # number of mc/sub-chunk PAIRS

                # ---- load: gather v rows + v_skip slab ----
                vg = vinp.tile([128, nsub, F_IN], F32, tag="vg")
                nc.gpsimd.dma_gather(vg[:], v_d.ap(),
                                     idx_sb[:, s0 * 8:(s0 + nsub) * 8],
                                     nt, nt, F_IN)
                vs = vinp.tile([128, nsub, F_IN], F32, tag="vs")
                nc.sync.dma_start(vs[:], vskip_ap[:, s0 * F_IN:(s0 + nsub) * F_IN])

                # ---- LayerNorm stats ----
                # S[:, a, c, g, 0:3] : a in {vg, vs}, c = sub-chunk,
                # g in {even, odd}; triple = (count, mean, count*var)
                S = stp.tile([128, 2, 4, 6], F32, tag="S")
                bns = []
                for c in range(nsub):
                    # HW BNStats handles exactly one 6-wide group per op
                    bns.append(nc.vector.bn_stats(S[:, 0, c, :], vg[:, c, :]))
                    bns.append(nc.vector.bn_stats(S[:, 1, c, :], vs[:, c, :]))
                # per sub-chunk: sum of 4 group means / count*vars over (a,g)
                red = stp.tile([128, 4, 4], F32, tag="red")  # rows: Sm, Sv, Sm2, -
                Sm = red[:, 0, 0:nsub]
                Sv = red[:, 1, 0:nsub]
                Sm2 = red[:, 2, 0:nsub]
                msc = stp.tile([128, 4, 2, 2], F32, tag="msc")  # squared means
                # view with (c) outer and (a,g) innermost to reduce over:
                m_view = S[:].rearrange("p a c (g x) -> p c a g x", g=2)
                _rr = [
                    nc.vector.tensor_reduce(Sm, m_view[:, 0:nsub, :, :, 1],
                                            AX.XY, OP.add),
                    nc.vector.tensor_reduce(Sv, m_view[:, 0:nsub, :, :, 2],
                                            AX.XY, OP.add),
                    nc.vector.tensor_tensor(msc[:, 0:nsub, :, :],
                                            m_view[:, 0:nsub, :, :, 1],
                                            m_view[:, 0:nsub, :, :, 1],
                                            OP.mult),
                ]
                _rr.append(nc.vector.tensor_reduce(Sm2, msc[:, 0:nsub, :, :],
                                                   AX.XY, OP.add))
                for _r in _rr:
                    for _b in bns:
                        tile.add_dep_helper(_r.ins, _b.ins, sync=True,
                                            reason="bn_stats -> strided reduce")

                r = e_sb[:, s0:s0 + nsub]
                st = stp.tile([128, 8, 4], F32, tag="st")
                t1, r2, t2, t3, t4, msq, muv, mu2 = (st[:, i, 0:nsub]
                                                     for i in range(8))
                st2 = stp.tile([128, 2, 4], F32, tag="st2")
                varv, lv = (st2[:, i, 0:nsub] for i in range(2))
                # mu = (-A1*r + A0)/1024 + Sm/8      (Sm = sum of 4 means)
                nc.vector.tensor_scalar(t1, r, csts[:, 0:1], csts[:, 1:2],
                                        OP.mult, OP.add)
                nc.vector.scalar_tensor_tensor(muv, Sm, 0.125, t1, OP.mult, OP.add)
                # msq = (B2 r^2 - 2 B1 r + B0)/1024 + Sv/1024 + Sm2/8
                nc.vector.tensor_mul(r2, r, r)
                nc.vector.tensor_scalar(t2, r2, csts[:, 2:3], csts[:, 4:5],
                                        OP.mult, OP.add)
                nc.vector.scalar_tensor_tensor(t3, r, csts[:, 3:4], t2,
                                               OP.mult, OP.add)
                nc.vector.scalar_tensor_tensor(t4, Sv, 1.0 / 128.0, Sm2,
                                               OP.mult, OP.add)
                nc.vector.scalar_tensor_tensor(msq, t4, 0.125, t3, OP.mult, OP.add)
                nc.vector.tensor_mul(mu2, muv, muv)
                nc.vector.tensor_sub(varv, msq, mu2)
                nc.scalar.activation(lv, varv, AF.Ln, bias=csts[:, 5:6])
                s4 = stp.tile([128, nsub, 4], F32, tag="s4")
                rstd = s4[:, :, 1]
                nc.scalar.activation(rstd, lv, AF.Exp, bias=0.0, scale=-0.5)
                nc.vector.scalar_tensor_tensor(s4[:, :, 0], r, -1.0, rstd,
                                               OP.mult, OP.mult)
                nc.vector.scalar_tensor_tensor(s4[:, :, 2], muv, -1.0, rstd,
                                               OP.mult, OP.mult)
                nc.vector.tensor_scalar(s4[:, :, 3], rstd, 0.0, 1.0,
                                        OP.mult, OP.add)

                # stats rows -> [4, nt] fp16 (transpose per sub-chunk)
                rhs4 = stp.tile([4, 512], F16, tag="rhs4")
                for c in range(nsub):
                    sps = xtps.tile([4, 128], F32, tag="xps")
                    nc.tensor.transpose(sps[:], s4[:, c, :], eye_f[:])
                    nc.vector.tensor_copy(rhs4[:, c * 128:(c + 1) * 128], sps[:])

                # ---- scale gathered features by rstd (per node) -> fp16 ----
                vgs = vscp.tile([128, nsub, F_IN], F16, tag="vgs")
                vss = vscp.tile([128, nsub, F_IN], F16, tag="vss")
                for c in range(nsub):
                    rs = s4[:, c, 1:2]
                    nc.vector.tensor_scalar(vgs[:, c, :], vg[:, c, :], rs, None,
                                            OP.mult)
                    nc.vector.tensor_scalar(vss[:, c, :], vs[:, c, :], rs, None,
                                            OP.mult)

                # ---- transpose to [feat, node] ----
                xts = []
                for kc in range(4):
                    src = vgs if kc < 2 else vss
                    off = (kc % 2) * 128
                    xps = xtps.tile([128, 512], F16, tag="xps")
                    for c in range(nsub):
                        nc.tensor.transpose(xps[:, c * 128:(c + 1) * 128],
                                            src[:, c, off:off + 128], eye_b[:])
                    xt = xtp.tile([128, 512], F16, tag=f"xt{kc}")
                    if kc < 3:
                        nc.scalar.copy(xt[:, 0:nt], xps[:, 0:nt])
                    else:
                        nc.vector.tensor_copy(xt[:, 0:nt], xps[:, 0:nt])
                    xts.append(xt)

                # ---- layers 0-2: [feat, node]; mc pairs share a PSUM tile ----
                def ff_layer(wmat, hsrcs, hpool, htag, lix):
                    """z = W @ hsrc (+ stats rows for layer 0), selu -> h tiles.
                    Returns list of 2 tiles [128, 2, 512] (mc pairs)."""
                    houts = []
                    for half in range(2):
                        z = zps.tile([128, 2, 512], F32, tag="z")
                        for i in range(2):
                            mc = 2 * half + i
                            for kc in range(4):
                                b = kc * 4 + mc
                                nc.tensor.matmul(
                                    z[:, i, 0:nt],
                                    wmat[:, b * 128:(b + 1) * 128],
                                    hsrcs[kc][:, 0:nt],
                                    start=(kc == 0),
                                    stop=(lix != 0 and kc == 3))
                            if lix == 0:
                                nc.tensor.matmul(z[:, i, 0:nt],
                                                 p4[:, mc * 128:(mc + 1) * 128],
                                                 rhs4[:, 0:nt],
                                                 start=False, stop=True)
                        e = erp.tile([128, 2, 512], F16, tag=f"e{lix}")
                        rl = erp.tile([128, 2, 512], F16, tag=f"r{lix}")
                        fused = (nt == 512 and lix == 0)
                        if fused:
                            zv = z[:, :, 0:nt].rearrange("p a n -> p (a n)")
                            ev = e[:, :, 0:nt].rearrange("p a n -> p (a n)")
                            rv = rl[:, :, 0:nt].rearrange("p a n -> p (a n)")
                            nc.scalar.activation(ev, zv, AF.Exp, bias=csts[:, 6:7])
                            nc.vector.tensor_scalar(rv, zv, 0.0, None, OP.max)
                        else:
                            bcol = biases[:, 8 * (lix - 1):] if lix else None
                            for i in range(2):
                                mc = 2 * half + i
                                if lix == 0:
                                    nc.scalar.activation(
                                        e[:, i, 0:nt], z[:, i, 0:nt], AF.Exp,
                                        bias=csts[:, 6:7])
                                    nc.vector.tensor_scalar(
                                        rl[:, i, 0:nt], z[:, i, 0:nt], 0.0,
                                        None, OP.max)
                                    continue
                                nc.scalar.activation(
                                    e[:, i, 0:nt], z[:, i, 0:nt], AF.Exp,
                                    bias=bcol[:, mc:mc + 1])
                                nc.scalar.activation(
                                    rl[:, i, 0:nt], z[:, i, 0:nt], AF.Relu,
                                    bias=bcol[:, 4 + mc:5 + mc])
                        h = hpool.tile([128, 2, 512], F16, tag=f"{htag}_{half}")
                        if nt == 512:
                            hv = h[:].rearrange("p a n -> p (a n)")
                            ev = e[:].rearrange("p a n -> p (a n)")
                            rv = rl[:].rearrange("p a n -> p (a n)")
                            nc.vector.scalar_tensor_tensor(hv, ev, ALPHA, rv,
                                                           OP.min, OP.add)
                        else:
                            for i in range(2):
                                nc.vector.scalar_tensor_tensor(
                                    h[:, i, 0:nt], e[:, i, 0:nt], ALPHA,
                                    rl[:, i, 0:nt], OP.min, OP.add)
                        houts.append(h)
                    return houts

                def hslice(hts, kc):
                    return hts[kc // 2][:, kc % 2, :]

                h1s = ff_layer(wv, xts, h1p, "h1", 0)
                h1v = [hslice(h1s, kc) for kc in range(4)]
                h2s = ff_layer(w1t, h1v, h2p, "h2", 1)
                h2v = [hslice(h2s, kc) for kc in range(4)]
                h3s = ff_layer(w2t, h2v, h3p, "h3", 2)
                h3v = [hslice(h3s, kc) for kc in range(4)]

                # ---- layer 3: stationary = h3 -> output [node, feat] ----
                for half in range(nh):
                    z = zps.tile([128, 2, 512], F32, tag="z")
                    for i in range(2):
                        c = 2 * half + i
                        for kc in range(4):
                            nc.tensor.matmul(z[:, i, :],
                                             h3v[kc][:, c * 128:(c + 1) * 128],
                                             w3t[:, kc * 512:(kc + 1) * 512],
                                             start=(kc == 0), stop=False)
                        nc.tensor.matmul(z[:, i, :], ones1[0:1, :], b3row[0:1, :],
                                         start=False, stop=True)
                    zv = z[:].rearrange("p a n -> p (a n)")
                    e = finp.tile([128, 1024], F32, tag="e3")
                    nc.scalar.activation(e[:], zv, AF.Exp, bias=csts[:, 7:8])
                    rl = finp.tile([128, 1024], F32, tag="r3")
                    nc.scalar.activation(rl[:], zv, AF.Relu, bias=0.0, scale=LAMBDA)
                    s3 = finp.tile([128, 1024], F32, tag="s3")
                    nc.vector.scalar_tensor_tensor(s3[:], e[:], LA, rl[:],
                                                   OP.min, OP.add)
                    ob = finp.tile([128, 2, 512], F32, tag="ob")
                    nc.scalar.activation(
                        ob[:].rearrange("p a n -> p (a n)"), s3[:], AF.Identity,
                        bias=csts[:, 8:9])
                    row0 = (s0 + 2 * half) * 128
                    dst = out_d.ap()[row0:row0 + 256, :].rearrange(
                        "(a p) f -> p a f", p=128)
                    nc.sync.dma_start(dst, ob[:])

    nc.compile()
    return nc


def prep_host(v, e_rel, idx_hr_to_lr, v_skip, We, be, ln_g, ln_b,
              W0, b0, W1, b1, W2, b2, W3, b3, nodes_pc=NODES_PC, n_cores=N_CORES):
    """Returns (shared_inputs, per_core_fn)."""
    f64 = np.float64
    hdt = np.float16  # must match device matmul dtype F16
    W0g = (np.asarray(W0, f64) * np.asarray(ln_g, f64)[None, :])
    WeC = np.asarray(We, f64)[:, 0]
    beC = np.asarray(be, f64)

    def blocks(M):  # [512,512] -> [128, 2048] with block b = kc*4+mc
        out = np.empty((128, 2048), f64)
        MT = M.T  # [in, out]
        for kc in range(4):
            for mc in range(4):
                b = kc * 4 + mc
                out[:, b * 128:(b + 1) * 128] = MT[kc * 128:(kc + 1) * 128,
                                                   mc * 128:(mc + 1) * 128]
        return out

    wv = blocks(W0g[:, 512:1024])
    w1t = blocks(LAMBDA * np.asarray(W1, f64))
    w2t = blocks(LAMBDA * np.asarray(W2, f64))
    w3tT = (LAMBDA * np.asarray(W3, f64)).T  # [in, out]
    w3t = np.concatenate([w3tT[k * 128:(k + 1) * 128, :] for k in range(4)], axis=1)

    p1 = W0g[:, :512] @ WeC
    p0 = W0g[:, :512] @ beC
    q = W0g.sum(1)
    c0 = np.asarray(b0, f64) + np.asarray(W0, f64) @ np.asarray(ln_b, f64)
    p4 = np.stack([p1, p0, q, c0])  # [4, 512]

    b1p = np.asarray(b1, f64) - LA * np.asarray(W1, f64).sum(1)
    b2p = np.asarray(b2, f64) - LA * np.asarray(W2, f64).sum(1)
    b3p = np.asarray(b3, f64) - LA * np.asarray(W3, f64).sum(1)
    biases = np.stack([b1p + math.log(ALPHA), b1p,
                       b2p + math.log(ALPHA), b2p], axis=0)  # [4(kind), 512]
    # -> [128, 16] with col = kind*4 + mc, partition p holds b[mc*128+p]
    biases = biases.reshape(4, 4, 128).transpose(2, 0, 1).reshape(128, 16)

    A1, A0 = WeC.sum(), beC.sum()
    B2, B1, B0 = (WeC ** 2).sum(), (WeC * beC).sum(), (beC ** 2).sum()
    csts = np.zeros((128, 12), f64)
    csts[:, 0] = -A1 / 1024.0
    csts[:, 1] = A0 / 1024.0
    csts[:, 2] = B2 / 1024.0
    csts[:, 3] = -2.0 * B1 / 1024.0
    csts[:, 4] = B0 / 1024.0
    csts[:, 5] = LN_EPS
    csts[:, 6] = math.log(ALPHA)
    csts[:, 7] = math.log(LA)
    csts[:, 8] = -LA

    shared = dict(
        v=np.ascontiguousarray(np.asarray(v, np.float32)),
        wv=wv.astype(hdt), w1t=w1t.astype(hdt), w2t=w2t.astype(hdt),
        w3t=w3t.astype(hdt), p4=p4.astype(hdt),
        b3row=b3p[None, :].astype(hdt),
        ones1=np.ones((1, 128), hdt),
        eye_b=np.eye(128, dtype=np.float32).astype(hdt),
        eye_f=np.eye(128, dtype=np.float32),
        biases=biases.astype(np.float32),
        csts=csts.astype(np.float32),
    )

    e_rel = np.asarray(e_rel, np.float32)
    idx_hr_to_lr = np.asarray(idx_hr_to_lr)
    v_skip = np.asarray(v_skip, np.float32)
    n_hr = e_rel.shape[0]
    per = n_hr // n_cores

    def per_core(core):
        lo = core * per
        hi = lo + per
        npc = nodes_pc
        er = np.zeros(npc, np.float32)
        er[:hi - lo] = e_rel[lo:hi, 0]
        ix = np.zeros(npc, np.int16)
        ix[:hi - lo] = idx_hr_to_lr[lo:hi].astype(np.int16)
        vsk = np.zeros((npc, F_IN), np.float32)
        vsk[:hi - lo] = v_skip[lo:hi]
        m = dict(shared)
        m["erel_t"] = np.ascontiguousarray(er.reshape(npc // 128, 128).T)
        m["idx16"] = np.ascontiguousarray(np.tile(ix.reshape(npc // 16, 16).T, (8, 1)))
        m["vskip_t"] = np.ascontiguousarray(
            vsk.reshape(npc // 128, 128, F_IN).transpose(1, 0, 2).reshape(128, -1))
        return m

    return shared, per_core


_NC_CACHE = {}


def kernel(v, e_rel, idx_hr_to_lr, v_skip, We, be, ln_g, ln_b,
           W0, b0, W1, b1, W2, b2, W3, b3):
    key = ("full", NODES_PC)
    if key not in _NC_CACHE:
        _NC_CACHE[key] = build_nc()
    nc = _NC_CACHE[key]
    _, per_core = prep_host(v, e_rel, idx_hr_to_lr, v_skip, We, be, ln_g, ln_b,
                            W0, b0, W1, b1, W2, b2, W3, b3)
    in_maps = [per_core(c) for c in range(N_CORES)]
    res = run_bass_kernel_spmd(nc, in_maps, core_ids=list(range(N_CORES)))
    per = N_HR // N_CORES
    out = np.concatenate([res.results[c]["out"][:per] for c in range(N_CORES)], axis=0)
    return np.asarray(out, np.float32)
